# revision 1
# baseline (speedup 1.0000x reference)
"""DFine MultiScale Deformable Attention — Trainium2 Bass kernel.

Full inputs in, full outputs out. Data-parallel over batch: 32 batches
split 4-per-core across 8 NeuronCores (same SPMD program, per-core
input shards).

Per-core pipeline (per batch b):
  1. DMA value[b] [8400,256] f32 -> SBUF staged tiles; PE-transpose into
     two per-head-half tables T_half [128ch=(h,16d), 8448 pixels] f32.
  2. Frontend in transposed layout [(h,p)=96 part, q=336 free]:
     PE matmuls for offsets/attn-logits (lhsT = weights, rhs = query^T),
     softmax via ACT exp + PE column-sum + DVE reciprocal,
     bilinear coords/weights on DVE, sample indices -> int16.
  3. idx wrap: PE-transpose lin16 to q-partitions, ACT copies into the
     per-core wrapped index tile for ap_gather.
  4. GPSIMD ap_gather: G[128ch, i] = T_half[ch, idx_h(i)] (per-core idx
     streams = per-head sample lists, 48 samples x 336 q-slots).
  5. Weights broadcast across partitions via PE selector matmuls
     (W_p = SELP_p^T @ w4), DVE multiply + tree-reduce over (p, corner).
  6. PE-transpose result back to q-partitions, DMA to DRAM.
"""

import numpy as np

import concourse.bass as bass
import concourse.tile as tile
from concourse import bacc, mybir, library_config
from concourse.bass_utils import run_bass_kernel_spmd

F32 = mybir.dt.float32
I16 = mybir.dt.int16

# Problem constants (hardcoded per task spec)
B, LQ, DM, NH, HD = 32, 300, 256, 8, 32
NP_TOT = 12                      # points per (q, h)
LVL_W = [80, 40, 20]             # square levels
LVL_BASE = [0, 6400, 8000]
S = 8400
S_PAD = 8448                     # 66 * 128
N_CORES = 8
BPC = B // N_CORES               # batches per core
SHIFT = 64.0                     # floor-shift (positive before cast)
# HW float->int16 cast rounds to nearest; floor(x) == round(x - 0.5).
# CoreSim truncates instead; sim tests set CAST_BIAS = SHIFT - 0.5.
CAST_BIAS = SHIFT - 1.0
REPEAT = 1                       # benchmark: repeat the whole per-core program

# q layout: 3 slots of 112 partitions; q = 112*slot + r;  336 q-slots (300 real)
QP, NSLOT, Q336 = 112, 3, 336
# per-head sample list: i = 16*(slot*336 + k*48 + 4*p + t) + a
#   q = 112*slot + 16*k + a ; k in [0,7), a in [0,16)
NIDX = 16 * 7 * 3 * 48           # 16128 per head list
NIDX_SLOT = NIDX // 3            # 5376
IDXC = NIDX // 16                # 1008 cols in idx tile
IDXC_SLOT = IDXC // 3            # 336


def _wl(p):
    return float(LVL_W[p // 4])


def _base(p):
    return float(LVL_BASE[p // 4])


def make_consts():
    c = {}
    c["ident"] = np.eye(128, dtype=np.float32)
    # SELP: 24 matrices [96, 128] (g-major); SELP[g,p][(h,p'), ch] =
    # (p'==p && h == 4g + ch//32)
    selp = np.zeros((96, 2 * 12 * 128), np.float32)
    for g in range(2):
        for p in range(12):
            for hl in range(4):
                for d in range(32):
                    selp[(4 * g + hl) * 12 + p,
                         (g * 12 + p) * 128 + hl * 32 + d] = 1.0
    c["selp"] = selp
    sel8 = np.zeros((96, 8), np.float32)
    rep8 = np.zeros((8, 96), np.float32)
    for h in range(8):
        for p in range(12):
            sel8[h * 12 + p, h] = 1.0
            rep8[h, h * 12 + p] = 1.0
    c["sel8"] = sel8
    c["rep8"] = rep8
    # SELR: 4 matrices [4, 96]: rows of refT = (x, y, w, h)
    selr = np.zeros((4, 4 * 96), np.float32)
    for hp in range(96):
        p = hp % 12
        w = _wl(p)
        selr[0, 0 * 96 + hp] = w            # RXW: ref_x * W
        selr[1, 1 * 96 + hp] = w            # RYW: ref_y * W
        selr[2, 2 * 96 + hp] = 0.125 * w    # RWC: ref_w * pscale*0.5*W
        selr[3, 3 * 96 + hp] = 0.125 * w    # RHC
    c["selr"] = selr
    c["ones1"] = np.ones((1, Q336), np.float32)
    # per-partition const vectors [96, 8]
    cv = np.zeros((96, 8), np.float32)
    for hp in range(96):
        p = hp % 12
        w, base = _wl(p), _base(p)
        cv[hp, 0] = w - 1.0 + SHIFT          # XMAX  (clip for x0/y0)
        cv[hp, 1] = w - 2.0 + SHIFT          # XM63  (clip for x1/y1 pre +1)
        cv[hp, 2] = w                        # Wv
        cv[hp, 3] = base - SHIFT * w - SHIFT          # Cl00
        cv[hp, 4] = base - SHIFT * w - SHIFT + 1.0    # Cl01
        cv[hp, 5] = base - (SHIFT - 1.0) * w - SHIFT  # Cl10
        cv[hp, 6] = base - (SHIFT - 1.0) * w - SHIFT + 1.0  # Cl11
    c["cv"] = cv
    return c


def emit(nc):
    """Emit the full per-core program (BPC batches)."""
    # ---- DRAM tensors -------------------------------------------------
    value = nc.dram_tensor("value", [BPC, S, DM], F32, kind="ExternalInput").ap()
    query = nc.dram_tensor("query", [BPC, LQ, DM], F32, kind="ExternalInput").ap()
    refp = nc.dram_tensor("refp", [BPC, LQ, 4], F32, kind="ExternalInput").ap()
    woff = nc.dram_tensor("woff", [DM, 192], F32, kind="ExternalInput").ap()
    wattn = nc.dram_tensor("wattn", [DM, 96], F32, kind="ExternalInput").ap()
    boff = nc.dram_tensor("boff", [1, 192], F32, kind="ExternalInput").ap()
    battn = nc.dram_tensor("battn", [1, 96], F32, kind="ExternalInput").ap()
    ident_d = nc.dram_tensor("ident", [128, 128], F32, kind="ExternalInput").ap()
    selp_d = nc.dram_tensor("selp", [96, 24 * 128], F32, kind="ExternalInput").ap()
    sel8_d = nc.dram_tensor("sel8", [96, 8], F32, kind="ExternalInput").ap()
    rep8_d = nc.dram_tensor("rep8", [8, 96], F32, kind="ExternalInput").ap()
    selr_d = nc.dram_tensor("selr", [4, 4 * 96], F32, kind="ExternalInput").ap()
    ones1_d = nc.dram_tensor("ones1", [1, Q336], F32, kind="ExternalInput").ap()
    cv_d = nc.dram_tensor("cv", [96, 8], F32, kind="ExternalInput").ap()
    out_d = nc.dram_tensor("out", [BPC, LQ, DM], F32, kind="ExternalOutput").ap()
    linq_d = nc.dram_tensor("linq", [BPC, 2, QP, NSLOT * 8 * 48], I16,
                            kind="Internal").ap()

    MUL, ADD, SUB, MAX, MIN, EQ = (
        mybir.AluOpType.mult, mybir.AluOpType.add, mybir.AluOpType.subtract,
        mybir.AluOpType.max, mybir.AluOpType.min, mybir.AluOpType.is_equal)
    EXP = mybir.ActivationFunctionType.Exp

    with tile.TileContext(nc) as tc:
        import contextlib
        ctx = contextlib.ExitStack()
        with ctx:
            cpool = ctx.enter_context(tc.tile_pool(name="consts", bufs=1))
            spool = ctx.enter_context(tc.tile_pool(name="staged", bufs=2))
            tpool = ctx.enter_context(tc.tile_pool(name="tables", bufs=1))
            gpool = ctx.enter_context(tc.tile_pool(name="gath", bufs=2))
            fpool = ctx.enter_context(tc.tile_pool(name="front", bufs=16))
            wpool = ctx.enter_context(tc.tile_pool(name="w4", bufs=1))
            ppool = ctx.enter_context(tc.tile_pool(name="ptree", bufs=2))
            opool = ctx.enter_context(tc.tile_pool(name="outsb", bufs=1))
            ipool = ctx.enter_context(tc.tile_pool(name="idx", bufs=2))
            qpool = ctx.enter_context(tc.tile_pool(name="qt", bufs=1))
            psT = ctx.enter_context(tc.tile_pool(name="psT", bufs=2, space="PSUM"))
            psS = ctx.enter_context(tc.tile_pool(name="psS", bufs=2, space="PSUM"))
            psW = ctx.enter_context(tc.tile_pool(name="psW", bufs=3, space="PSUM"))
            psF = ctx.enter_context(tc.tile_pool(name="psF", bufs=1, space="PSUM"))

            nc.gpsimd.load_library(library_config.ap_gather)

            # ---- load constants -----------------------------------------
            def ld(dst, src):
                nc.sync.dma_start(dst, src)

            ident = cpool.tile([128, 128], F32, name="ident")
            ld(ident[:], ident_d)
            selp = cpool.tile([96, 24 * 128], F32, name="selp")
            ld(selp[:], selp_d)
            sel8 = cpool.tile([96, 8], F32, name="sel8")
            ld(sel8[:], sel8_d)
            rep8 = cpool.tile([8, 96], F32, name="rep8")
            ld(rep8[:], rep8_d)
            selr = cpool.tile([4, 4 * 96], F32, name="selr")
            ld(selr[:], selr_d)
            ones1 = cpool.tile([1, Q336], F32, name="ones1")
            ld(ones1[:], ones1_d)
            cv = cpool.tile([96, 8], F32, name="cv")
            ld(cv[:], cv_d)
            woff_sb = cpool.tile([128, 2, 192], F32, name="woff_sb")
            ld(woff_sb[:], woff.rearrange("(kt p) m -> p kt m", p=128))
            wattn_sb = cpool.tile([128, 2, 96], F32, name="wattn_sb")
            ld(wattn_sb[:], wattn.rearrange("(kt p) m -> p kt m", p=128))
            boff_sb = cpool.tile([1, 192], F32, name="boff_sb")
            ld(boff_sb[:], boff)
            battn_sb = cpool.tile([1, 96], F32, name="battn_sb")
            ld(battn_sb[:], battn)

            def cvs(k):   # [96,1] scalar AP
                return cv[:, k:k + 1]

            def fs(nm):   # shared-tag frontend scratch [96, 336] f32
                return fpool.tile([96, Q336], F32, name=nm, tag="fs")

            for b4 in range(BPC * REPEAT):
                b = b4 % BPC
                # ==== 1. value staging + per-head-half tables =============
                Th = [tpool.tile([128, S_PAD], F32, name=f"T{h}", tag=f"T{h}")
                      for h in range(2)]
                # [128, 65, 256] view of the first 8320 pixels
                vb = value[b][0:8320].rearrange("(so sp) ch -> sp so ch", sp=128)

                def load_piece(j):
                    if j < 8:
                        pc = spool.tile([128, 8, 256], F32, name="piece", tag="piece")
                        nc.sync.dma_start(pc[:], vb[:, 8 * j:8 * j + 8, :])
                    else:
                        pc = spool.tile([128, 8, 256], F32, name="piece", tag="piece")
                        nc.sync.dma_start(pc[:, 0, :], vb[:, 64, :])
                        nc.vector.memset(pc[64:128, 1, :], 0.0)
                        nc.sync.dma_start(pc[0:80, 1, :], value[b][8320:8400, :])
                    return pc

                pieces = {}
                for g in range(17):           # groups of 4 so (66 = 16*4+2)
                    gsz = 4 if g < 16 else 2
                    for half in range(2):
                        pt = psT.tile([128, 512], F32, name="tpose", tag="tpose")
                        for u in range(gsz):
                            so = g * 4 + u
                            j = so // 8 if so < 64 else 8
                            if j not in pieces:
                                pieces[j] = load_piece(j)
                            loc = so % 8 if so < 64 else so - 64
                            src = pieces[j][:, loc,
                                            half * 128:(half + 1) * 128]
                            nc.tensor.transpose(pt[:, u * 128:(u + 1) * 128],
                                                src, ident[:])
                        nc.scalar.copy(Th[half][:, g * 512: g * 512 + gsz * 128],
                                       pt[:, 0:gsz * 128])

                # ==== 2. frontend ========================================
                qsb = qpool.tile([QP, NSLOT, DM], F32, name="qsb")
                nc.vector.memset(qsb[64:112, 2, :], 0.0)
                nc.sync.dma_start(
                    qsb[:, 0:2, :],
                    query[b][0:224].rearrange("(s r) c -> r s c", r=QP))
                nc.sync.dma_start(qsb[0:76, 2, :], query[b][224:300, :])
                refsb = qpool.tile([QP, NSLOT, 4], F32, name="refsb")
                nc.vector.memset(refsb[64:112, 2, :], 0.0)
                nc.sync.dma_start(
                    refsb[:, 0:2, :],
                    refp[b][0:224].rearrange("(s r) c -> r s c", r=QP))
                nc.sync.dma_start(refsb[0:76, 2, :], refp[b][224:300, :])

                # query^T [2][128, 336]
                qT = [qpool.tile([128, Q336], F32, name=f"qT{kt}") for kt in range(2)]
                for slot in range(NSLOT):
                    for kt in range(2):
                        pt = psS.tile([128, 128], F32, name="pss", tag="pss")
                        nc.tensor.transpose(
                            pt[:, 0:QP], qsb[:, slot, kt * 128:(kt + 1) * 128],
                            ident[0:QP, 0:QP])
                        nc.scalar.copy(qT[kt][:, slot * QP:(slot + 1) * QP],
                                       pt[:, 0:QP])
                refT = fs("refT")
                for slot in range(NSLOT):
                    pt = psS.tile([128, 128], F32, name="pss", tag="pss")
                    nc.tensor.transpose(pt[0:4, 0:QP], refsb[:, slot, :],
                                        ident[0:QP, 0:QP])
                    nc.scalar.copy(refT[0:4, slot * QP:(slot + 1) * QP],
                                   pt[0:4, 0:QP])

                # ref broadcasts: rxw, ryw, rwc, rhc  [96, 336]
                refb = []
                for m in range(4):
                    ps = psF.tile([96, Q336], F32, name="psf", tag="psf")
                    nc.tensor.matmul(ps[:], selr[:, m * 96:(m + 1) * 96], refT[0:4, :])
                    sb = fs(f"refb{m}")
                    nc.scalar.copy(sb[:], ps[:])
                    refb.append(sb)
                rxw, ryw, rwc, rhc = refb

                def head_mm(w_sb, bias_sb, cols, nm):
                    ps = psF.tile([96, Q336], F32, name="psf", tag="psf")
                    nc.tensor.matmul(ps[:], w_sb[:, 0, cols], qT[0][:],
                                     start=True, stop=False)
                    nc.tensor.matmul(ps[:], w_sb[:, 1, cols], qT[1][:],
                                     start=False, stop=False)
                    nc.tensor.matmul(ps[:], bias_sb[:, cols], ones1[:],
                                     start=False, stop=True)
                    return ps

                # attention softmax
                logit_ps = head_mm(wattn_sb, battn_sb, slice(0, 96), "logit")
                expT = fs("expT")
                nc.scalar.activation(expT[:], logit_ps[:], EXP)
                s_ps = psF.tile([96, Q336], F32, name="psf", tag="psf")
                nc.tensor.matmul(s_ps[0:8, :], sel8[:], expT[:])
                rsum = fs("rsum")
                nc.vector.reciprocal(rsum[0:8, :], s_ps[0:8, :])
                rb_ps = psF.tile([96, Q336], F32, name="psf", tag="psf")
                nc.tensor.matmul(rb_ps[:], rep8[:], rsum[0:8, :])
                attnT = fs("attnT")
                nc.vector.tensor_tensor(attnT[:], expT[:], rb_ps[:], MUL)

                offx_ps = head_mm(woff_sb, boff_sb, slice(0, 192, 2), "offx")
                offx = fs("offx")
                nc.scalar.copy(offx[:], offx_ps[:])
                offy_ps = head_mm(woff_sb, boff_sb, slice(1, 192, 2), "offy")
                offy = fs("offy")
                nc.scalar.copy(offy[:], offy_ps[:])

                # coords -> clipped ints, fracs, valids, weights
                def coord(off_sb, rXw, rWc, sfx):
                    t1 = fs("t1" + sfx)
                    nc.vector.tensor_tensor(t1[:], off_sb[:], rWc[:], MUL)
                    ixp = fs("ixp" + sfx)
                    nc.vector.scalar_tensor_tensor(
                        ixp[:], t1[:], CAST_BIAS, rXw[:], ADD, ADD)
                    fi = fpool.tile([96, Q336], I16, name="fi" + sfx, tag="fi")
                    nc.vector.tensor_copy(fi[:], ixp[:])
                    fxp = fs("fxp" + sfx)
                    nc.vector.tensor_copy(fxp[:], fi[:])
                    a0 = fs("a0" + sfx)
                    nc.vector.tensor_scalar(a0[:], fxp[:], SHIFT, cvs(0), MAX, MIN)
                    a1 = fs("a1" + sfx)
                    nc.vector.tensor_scalar(a1[:], fxp[:], SHIFT - 1.0, cvs(1), MAX, MIN)
                    v0 = fs("v0" + sfx)
                    nc.vector.tensor_tensor(v0[:], a0[:], fxp[:], EQ)
                    v1 = fs("v1" + sfx)
                    nc.vector.tensor_tensor(v1[:], a1[:], fxp[:], EQ)
                    fx = fs("fx" + sfx)
                    nc.vector.scalar_tensor_tensor(
                        fx[:], ixp[:], SHIFT - CAST_BIAS - 0.5, fxp[:], ADD, SUB)
                    omf = fs("omf" + sfx)
                    nc.vector.tensor_scalar(omf[:], fx[:], -1.0, 1.0, MUL, ADD)
                    w0 = fs("w0" + sfx)
                    nc.vector.tensor_tensor(w0[:], omf[:], v0[:], MUL)
                    w1 = fs("w1" + sfx)
                    nc.vector.tensor_tensor(w1[:], fx[:], v1[:], MUL)
                    return a0, a1, w0, w1

                a0x, a1x, wx0, wx1 = coord(offx, rxw, rwc, "x")
                a0y, a1y, wy0, wy1 = coord(offy, ryw, rhc, "y")

                wy0a = fs("wy0a")
                nc.vector.tensor_tensor(wy0a[:], wy0[:], attnT[:], MUL)
                wy1a = fs("wy1a")
                nc.vector.tensor_tensor(wy1a[:], wy1[:], attnT[:], MUL)

                # w4 [96, 4, 336]: t = 2*ty + tx
                w4 = wpool.tile([96, 4, Q336], F32, name="w4")
                nc.vector.tensor_tensor(w4[:, 0, :], wy0a[:], wx0[:], MUL)
                nc.vector.tensor_tensor(w4[:, 1, :], wy0a[:], wx1[:], MUL)
                nc.vector.tensor_tensor(w4[:, 2, :], wy1a[:], wx0[:], MUL)
                nc.vector.tensor_tensor(w4[:, 3, :], wy1a[:], wx1[:], MUL)

                # lin indices per corner -> transpose -> lin16q (int16)
                yw0 = fs("yw0")
                nc.vector.tensor_scalar(yw0[:], a0y[:], cvs(2), None, MUL)
                yw1 = fs("yw1")
                nc.vector.tensor_scalar(yw1[:], a1y[:], cvs(2), None, MUL)
                lin16q = ipool.tile([QP, NSLOT, 8, 12, 4], I16, name="lin16q",
                                    bufs=1)
                corner_spec = ((a0x, 3, yw0), (a1x, 4, yw0), (a0x, 5, yw1), (a1x, 6, yw1))
                for t, (ax, cl, yw) in enumerate(corner_spec):
                    lf = fs("linf")
                    nc.vector.scalar_tensor_tensor(lf[:], ax[:], cvs(cl), yw[:],
                                                   ADD, ADD)
                    for slot in range(NSLOT):
                        pt = psS.tile([128, 128], F32, name="pss", tag="pss")
                        nc.tensor.transpose(
                            pt[0:QP, 0:96], lf[:, slot * QP:(slot + 1) * QP],
                            ident[0:96, 0:96])
                        dst = lin16q[:, slot, :, :, t].rearrange("r h p -> r (h p)")
                        nc.scalar.copy(dst, pt[0:QP, 0:96])

                # wrapped idx tiles (one per head-group) via DRAM round-trip
                # (engines cannot address 16-aligned partition starts; DRAM is
                # flat).  Core j of group g serves head 4g + j//2, so each
                # head's list is duplicated across its core pair (hq = 2*hl+dup).
                idxt = []
                for g in range(2):
                    l2 = ipool.tile([QP, NSLOT, 8, 48], I16, name=f"l2_{g}",
                                    tag="l2", bufs=1)
                    for dup in range(2):
                        nc.scalar.copy(
                            l2[:, :, dup::2, :],
                            lin16q[:, :, 4 * g:4 * g + 4, :].rearrange(
                                "r sl h p t -> r sl h (p t)"))
                    nc.sync.dma_start(
                        linq_d[b, g], l2[:].rearrange("r sl h c -> r (sl h c)"))
                    it = ipool.tile([128, IDXC], I16, name=f"idxt{g}", tag="idxt")
                    # addr(r=16k+a, sl, hq, c); dst[16hq+a, sl*336+k*48+c]
                    lsrc = linq_d[b, g].rearrange(
                        "(k a) (sl hq c) -> hq a k sl c", a=16, sl=NSLOT, hq=8)
                    for sl in range(NSLOT):
                        for k in range(7):
                            nc.sync.dma_start(
                                it[:, sl * IDXC_SLOT + k * 48:
                                   sl * IDXC_SLOT + (k + 1) * 48],
                                lsrc[:, :, k, sl, :])
                    idxt.append(it)

                # ==== 4+5. gather + weighted reduce ======================
                oh = [opool.tile([128, Q336], F32, name=f"oh{g}")
                      for g in range(2)]
                for slot in range(NSLOT):
                    Gs = []
                    for g in range(2):
                        G = gpool.tile([128, NIDX_SLOT], F32, name="G", tag="G")
                        nc.gpsimd.ap_gather(
                            G[:], Th[g][:],
                            idxt[g][:, slot * IDXC_SLOT:(slot + 1) * IDXC_SLOT],
                            channels=128, num_elems=S_PAD, d=1,
                            num_idxs=NIDX_SLOT)
                        Gs.append(G)
                    rhs = w4[:, :, slot * QP:(slot + 1) * QP].rearrange(
                        "c t (k a) -> c k t a", k=7)
                    accs = [None, None]
                    # consume each group's G fully before the next so its
                    # buffer frees at mid-slot and the next ap_gather overlaps
                    # the remaining multiply work
                    for g in range(2):
                        for p in range(12):
                            wp = psW.tile([128, 7 * 64], F32, name="wp", tag="wp")
                            nc.tensor.matmul(
                                wp[:], selp[:, (g * 12 + p) * 128:
                                            (g * 12 + p + 1) * 128], rhs)
                            gv = Gs[g][:].rearrange(
                                "c (k p ta) -> c k p ta", k=7, p=12)[:, :, p, :]
                            wpv = wp[:].rearrange("c (k ta) -> c k ta", k=7)
                            nc.vector.tensor_tensor(gv, gv, wpv, MUL)  # in-place
                            if p == 0:
                                accs[g] = gv
                            elif p == 1:
                                na = ppool.tile([128, 7, 64], F32, name="acc",
                                                tag="acc")
                                nc.vector.tensor_tensor(na[:], accs[g], gv, ADD)
                                accs[g] = na
                            else:
                                nc.vector.tensor_tensor(accs[g][:], accs[g][:],
                                                        gv, ADD)
                    for g in range(2):
                        tot = accs[g]
                        x2 = ppool.tile([128, 7, 2, 16], F32, name="x2", tag="x2")
                        nc.vector.tensor_tensor(
                            x2[:], tot[:].rearrange("c k (t a) -> c k t a", t=4)[:, :, 0:2, :],
                            tot[:].rearrange("c k (t a) -> c k t a", t=4)[:, :, 2:4, :],
                            ADD)
                        nc.vector.tensor_tensor(
                            oh[g][:, slot * QP:(slot + 1) * QP].rearrange(
                                "c (k a) -> c k a", k=7),
                            x2[:, :, 0, :], x2[:, :, 1, :], ADD)

                # ==== 6. output transpose + store ========================
                osb = opool.tile([QP, NSLOT, DM], F32, name="osb")
                for g in range(2):
                    for slot in range(NSLOT):
                        pt = psS.tile([128, 128], F32, name="pss", tag="pss")
                        nc.tensor.transpose(
                            pt[0:QP, :], oh[g][:, slot * QP:(slot + 1) * QP],
                            ident[:])
                        nc.scalar.copy(
                            osb[:, slot, g * 128:(g + 1) * 128], pt[0:QP, :])

                nc.sync.dma_start(
                    out_d[b][0:224].rearrange("(s r) c -> r s c", r=QP),
                    osb[:, 0:2, :])
                nc.sync.dma_start(out_d[b][224:300, :], osb[0:76, 2, :])
    return nc


_CACHE = {}


def _get_nc():
    key = ("nc", REPEAT)
    if key not in _CACHE:
        nc = bacc.Bacc("TRN2", target_bir_lowering=False, debug=False,
                       enable_asserts=False)
        emit(nc)
        nc.compile()
        _CACHE[key] = nc
    return _CACHE[key]


def _in_maps(query, reference_points, input_flatten, W_off, b_off, W_attn,
             b_attn):
    query = np.asarray(query, np.float32)
    refp = np.asarray(reference_points, np.float32).reshape(B, LQ, 4)
    value = np.asarray(input_flatten, np.float32)
    consts = make_consts()
    in_maps = []
    for c in range(N_CORES):
        sl = slice(c * BPC, (c + 1) * BPC)
        in_maps.append({
            "value": value[sl], "query": query[sl], "refp": refp[sl],
            "woff": np.asarray(W_off, np.float32),
            "wattn": np.asarray(W_attn, np.float32),
            "boff": np.asarray(b_off, np.float32).reshape(1, 192),
            "battn": np.asarray(b_attn, np.float32).reshape(1, 96),
            **{k: v for k, v in consts.items()},
        })
    return in_maps


def kernel(query, reference_points, input_flatten, W_off, b_off, W_attn, b_attn):
    in_maps = _in_maps(query, reference_points, input_flatten, W_off, b_off,
                       W_attn, b_attn)
    res = run_bass_kernel_spmd(_get_nc(), in_maps, core_ids=list(range(N_CORES)))
    out = np.concatenate([r["out"] for r in res.results], axis=0)
    return out.astype(np.float32)


def kernel_traced(query, reference_points, input_flatten, W_off, b_off, W_attn,
                  b_attn):
    """Run with NTFF tracing; returns exec_time_ns (or None)."""
    in_maps = _in_maps(query, reference_points, input_flatten, W_off, b_off,
                       W_attn, b_attn)
    res = run_bass_kernel_spmd(_get_nc(), in_maps, core_ids=list(range(N_CORES)),
                               trace=True)
    print("trace:", res.instructions_and_trace[1] if res.instructions_and_trace else None)
    return res.exec_time_ns


if __name__ == "__main__":
    import reference
    inputs = reference.setup_inputs()
    inputs = {k: np.asarray(v) for k, v in inputs.items()}
    got = kernel(**inputs)
    exp = np.asarray(reference.reference(**inputs))
    err = np.abs(got - exp).max() / np.abs(exp).max()
    print("Relative error:", err)



# revision 15
# speedup vs baseline: 3.2932x; 3.2932x over previous
"""DFine MultiScale Deformable Attention — Trainium2 Bass kernel (v2).

Full inputs in, full outputs out. Data-parallel over batch: 32 batches
split 4-per-core across 8 NeuronCores (same SPMD program, per-core
input shards).

v2 redesign vs v1: the bottleneck was GPSIMD ap_gather read-command
overhead (~102 cyc per 4-index command, ReadOverlap=0 on trn2).  v1
gathered one f32 per (query, point, corner) index: 129K indices/batch.
v2 packs the value table as bf16 with d=4 per entry = (x-pair, d_lo):
  T[ch2=(8h,16dhi), px', xp, dlo] = V[px'-1+xp, h*32+dhi*2+dlo]
so ONE index serves a (q, point, y-corner): both x-corners and both
low d-channels arrive in one 8-byte fetch.  32256 indices/batch (4x
fewer commands, 4x more payload per command).  One table serves all 8
heads (Q7 core h owns head h's 16 partitions).

Pipeline per batch:
  1. DMA value [8400,256] f32 -> SBUF; PE-transpose with stride-2
     column selection (dlo split); strided bf16 copies into T with the
     x-pair duplication (entry px' holds pixels px'-1 and px').
  2. Frontend in transposed layout [(h,p)=96 part, q=336 free]:
     offsets/attn matmuls, softmax, bilinear coords.  Per (q,h,p) TWO
     gather indices (y0, y1 rows): px' = base + yc*W + x0c + 1, with
     x0c in [-1, W-1] (left-edge handled by the +1 table shift).
  3. idx wrap via DRAM round-trip -> IT[128=(h,a16), slot, yc, p, k].
  4. Per slot: ONE ap_gather (num_idxs=2688, d=4) -> G[128, yc, p, k,
     a, (xp,dlo)] bf16.
  5. Weights: W8[96,(yc,q,xp,dlo)] = attn*wy*wx; per (yc,p) PE
     broadcast (seld) to [(h,dhi), k,a,xp,dlo]; DVE multiply-accum
     over (yc,p); xp-fold -> oh[128=(h,dhi), q, dlo].
  6. PE-transpose back to q-partitions (per dlo, stride-2 merge), DMA.
"""

import numpy as np

import concourse.bass as bass
import concourse.tile as tile
from concourse import bacc, mybir, library_config
from concourse.bass_utils import run_bass_kernel_spmd

F32 = mybir.dt.float32
BF16 = mybir.dt.bfloat16
I16 = mybir.dt.int16

# Problem constants (hardcoded per task spec)
B, LQ, DM, NH, HD = 32, 300, 256, 8, 32
NP_TOT = 12                      # points per (q, h)
LVL_W = [80, 40, 20]             # square levels
LVL_BASE = [0, 6400, 8000]
S = 8400
S_PAD = 8448                     # 66 * 128
TBL_N = 8464                     # table entries (px' in [0, 8448]), padded
N_CORES = 8
BPC = B // N_CORES               # batches per core
SHIFT = 64.0                     # floor-shift (positive before cast)
CAST_BIAS = SHIFT - 1.0          # HW float->int16 rounds to nearest

# q layout: 3 slots of 112 partitions; q = 112*slot + r;  336 q-slots (300 real)
QP, NSLOT, Q336 = 112, 3, 336
NIDX_SLOT = 2 * 12 * 7 * 16      # 2688 indices per slot (yc, p, k, a)
IDXC_SLOT = NIDX_SLOT // 16      # 168 idx cols per slot


def _wl(p):
    return float(LVL_W[p // 4])


def _base(p):
    return float(LVL_BASE[p // 4])


def make_consts():
    c = {}
    c["ident"] = np.eye(128, dtype=np.float32)
    # SELD: 12 matrices [96, 128] (p-major); SELD_p[(h',p'), h*16+dhi] =
    # (p'==p && h'==h)
    seld = np.zeros((96, 12 * 128), np.float32)
    for h in range(8):
        for p in range(12):
            for dhi in range(16):
                seld[h * 12 + p, p * 128 + h * 16 + dhi] = 1.0
    c["seld"] = seld
    sel8 = np.zeros((96, 8), np.float32)
    rep8 = np.zeros((8, 96), np.float32)
    for h in range(8):
        for p in range(12):
            sel8[h * 12 + p, h] = 1.0
            rep8[h, h * 12 + p] = 1.0
    c["sel8"] = sel8
    c["rep8"] = rep8
    # SELR: 4 matrices [4, 96]: rows of refT = (x, y, w, h)
    selr = np.zeros((4, 4 * 96), np.float32)
    for hp in range(96):
        p = hp % 12
        w = _wl(p)
        selr[0, 0 * 96 + hp] = w            # RXW: ref_x * W
        selr[1, 1 * 96 + hp] = w            # RYW: ref_y * W
        selr[2, 2 * 96 + hp] = 0.125 * w    # RWC: ref_w * pscale*0.5*W
        selr[3, 3 * 96 + hp] = 0.125 * w    # RHC
    c["selr"] = selr
    c["ones1"] = np.ones((1, Q336), np.float32)
    # per-partition const vectors [96, 8]
    cv = np.zeros((96, 8), np.float32)
    for hp in range(96):
        p = hp % 12
        w, base = _wl(p), _base(p)
        cv[hp, 0] = w - 1.0 + SHIFT          # XMAX  (clip for x0/y0)
        cv[hp, 1] = w - 2.0 + SHIFT          # XM63  (clip for x1/y1 pre +1)
        cv[hp, 2] = w                        # Wv
        # lin0 = xcl + C0 + a0y*W ; px' = base + y0c*W + x0c + 1
        cv[hp, 3] = base + 1.0 - SHIFT * w - SHIFT           # C0
        cv[hp, 4] = base + 1.0 + w - SHIFT * w - SHIFT       # C1 (y1 = a1y+1)
        # lvl12 y-merged: e = (y+1)*W + x0c + 1 = ycl*W + xcl + C2
        cv[hp, 5] = w - SHIFT * w - SHIFT + 1.0              # C2
    c["cv"] = cv
    return c


def emit(nc, repeat=1):
    """Emit the full per-core program (BPC batches, x repeat)."""
    # ---- DRAM tensors -------------------------------------------------
    value = nc.dram_tensor("value", [BPC, S, DM], F32, kind="ExternalInput").ap()
    query = nc.dram_tensor("query", [BPC, LQ, DM], F32, kind="ExternalInput").ap()
    refp = nc.dram_tensor("refp", [BPC, LQ, 4], F32, kind="ExternalInput").ap()
    woff = nc.dram_tensor("woff", [DM, 192], F32, kind="ExternalInput").ap()
    wattn = nc.dram_tensor("wattn", [DM, 96], F32, kind="ExternalInput").ap()
    boff = nc.dram_tensor("boff", [1, 192], F32, kind="ExternalInput").ap()
    battn = nc.dram_tensor("battn", [1, 96], F32, kind="ExternalInput").ap()
    ident_d = nc.dram_tensor("ident", [128, 128], F32, kind="ExternalInput").ap()
    seld_d = nc.dram_tensor("seld", [96, 12 * 128], F32, kind="ExternalInput").ap()
    sel8_d = nc.dram_tensor("sel8", [96, 8], F32, kind="ExternalInput").ap()
    rep8_d = nc.dram_tensor("rep8", [8, 96], F32, kind="ExternalInput").ap()
    selr_d = nc.dram_tensor("selr", [4, 4 * 96], F32, kind="ExternalInput").ap()
    ones1_d = nc.dram_tensor("ones1", [1, Q336], F32, kind="ExternalInput").ap()
    cv_d = nc.dram_tensor("cv", [96, 8], F32, kind="ExternalInput").ap()
    out_d = nc.dram_tensor("out", [BPC, LQ, DM], F32, kind="ExternalOutput").ap()
    linq_d = nc.dram_tensor("linq", [BPC, QP, NSLOT * 8 * 16], I16,
                            kind="Internal").ap()

    MUL, ADD, SUB, MAX, MIN, EQ = (
        mybir.AluOpType.mult, mybir.AluOpType.add, mybir.AluOpType.subtract,
        mybir.AluOpType.max, mybir.AluOpType.min, mybir.AluOpType.is_equal)
    EXP = mybir.ActivationFunctionType.Exp

    with tile.TileContext(nc) as tc:
        import contextlib
        ctx = contextlib.ExitStack()
        with ctx:
            cpool = ctx.enter_context(tc.tile_pool(name="consts", bufs=1))
            spool = ctx.enter_context(tc.tile_pool(name="staged", bufs=1))
            tpool = ctx.enter_context(tc.tile_pool(name="tables", bufs=1))
            gpool = ctx.enter_context(tc.tile_pool(name="gath", bufs=2))
            fpool = ctx.enter_context(tc.tile_pool(name="front", bufs=13))
            wpool = ctx.enter_context(tc.tile_pool(name="w8", bufs=1))
            ppool = ctx.enter_context(tc.tile_pool(name="ptree", bufs=2))
            opool = ctx.enter_context(tc.tile_pool(name="outsb", bufs=1))
            ipool = ctx.enter_context(tc.tile_pool(name="idx", bufs=2))
            qpool = ctx.enter_context(tc.tile_pool(name="qt", bufs=1))
            psT = ctx.enter_context(tc.tile_pool(name="psT", bufs=3, space="PSUM"))
            psS = ctx.enter_context(tc.tile_pool(name="psS", bufs=2, space="PSUM"))
            psW = ctx.enter_context(tc.tile_pool(name="psW", bufs=2, space="PSUM"))
            psF = ctx.enter_context(tc.tile_pool(name="psF", bufs=1, space="PSUM"))

            nc.gpsimd.load_library(library_config.ap_gather)

            # ---- load constants -----------------------------------------
            def ld(dst, src):
                nc.sync.dma_start(dst, src)

            ident = cpool.tile([128, 128], F32, name="ident")
            ld(ident[:], ident_d)
            seld = cpool.tile([96, 12 * 128], F32, name="seld")
            ld(seld[:], seld_d)
            sel8 = cpool.tile([96, 8], F32, name="sel8")
            ld(sel8[:], sel8_d)
            rep8 = cpool.tile([8, 96], F32, name="rep8")
            ld(rep8[:], rep8_d)
            selr = cpool.tile([4, 4 * 96], F32, name="selr")
            ld(selr[:], selr_d)
            ones1 = cpool.tile([1, Q336], F32, name="ones1")
            ld(ones1[:], ones1_d)
            cv = cpool.tile([96, 8], F32, name="cv")
            ld(cv[:], cv_d)
            woff_sb = cpool.tile([128, 2, 192], F32, name="woff_sb")
            ld(woff_sb[:], woff.rearrange("(kt p) m -> p kt m", p=128))
            wattn_sb = cpool.tile([128, 2, 96], F32, name="wattn_sb")
            ld(wattn_sb[:], wattn.rearrange("(kt p) m -> p kt m", p=128))
            boff_sb = cpool.tile([1, 192], F32, name="boff_sb")
            ld(boff_sb[:], boff)
            battn_sb = cpool.tile([1, 96], F32, name="battn_sb")
            ld(battn_sb[:], battn)

            def cvs(k):   # [96,1] scalar AP
                return cv[:, k:k + 1]

            def fs(nm):   # shared-tag frontend scratch [96, 336] f32
                return fpool.tile([96, Q336], F32, name=nm, tag="fs")

            for b4 in range(BPC * repeat):
                b = b4 % BPC
                # ==== 1. value staging -> packed bf16 table ==============
                # T[ch2, px'*4 + xp*2 + dlo] = V[px'-1+xp, h*32+dhi*2+dlo]
                T = tpool.tile([128, TBL_N * 4], BF16, name="T", tag="T")
                nc.vector.memset(T[:, 0:2], 0.0)   # entry 0, xp=0 slots
                # [128, 65, 256] view of the first 8320 pixels
                vb = value[b][0:8320].rearrange("(so sp) ch -> sp so ch", sp=128)

                def load_piece(j):
                    if j < 8:
                        pc = spool.tile([128, 8, 256], F32, name="piece", tag="piece")
                        nc.sync.dma_start(pc[:], vb[:, 8 * j:8 * j + 8, :])
                    else:
                        pc = spool.tile([128, 8, 256], F32, name="piece", tag="piece")
                        nc.sync.dma_start(pc[:, 0, :], vb[:, 64, :])
                        nc.vector.memset(pc[64:128, 1, :], 0.0)
                        nc.sync.dma_start(pc[0:80, 1, :], value[b][8320:8400, :])
                    return pc

                pieces = {}
                for g in range(17):           # groups of 4 chunks (66 = 16*4+2)
                    gsz = 4 if g < 16 else 2
                    px0 = 512 * g
                    for dlo in range(2):
                        pt = psT.tile([128, 512], F32, name="tpose", tag="tpose")
                        for u in range(gsz):
                            so = g * 4 + u
                            j = so // 8 if so < 64 else 8
                            if j not in pieces:
                                pieces[j] = load_piece(j)
                            loc = so % 8 if so < 64 else so - 64
                            src = pieces[j][:, loc, dlo::2]
                            nc.tensor.transpose(pt[:, u * 128:(u + 1) * 128],
                                                src, ident[:])
                        n = gsz * 128
                        for xp in range(2):
                            st = 4 * px0 + (4 - 2 * xp) + dlo
                            dst = T[:, st:st + 4 * n:4]
                            if xp == 0:
                                nc.scalar.copy(dst, pt[:, 0:n])
                            else:
                                nc.vector.tensor_copy(dst, pt[:, 0:n])

                # ---- y-merged d=8 tables for levels 1, 2 (from T) -------
                # T1[e*8+yp*4+xp*2+dlo] = V[e+6359+40*yp+xp] (lvl1 pixel
                # (y+yp, x0c+xp), e=(y+1)*40+x0c+1); V[g] = T[g*4+2+dlo]
                T1 = tpool.tile([128, 1648 * 8], BF16, name="T1", tag="T1")
                T2 = tpool.tile([128, 432 * 8], BF16, name="T2", tag="T2")
                for yp in range(2):
                    for xp in range(2):
                        for dlo in range(2):
                            off = yp * 4 + xp * 2 + dlo
                            eng = nc.scalar.copy if yp == 0 else nc.vector.tensor_copy
                            st1 = (6359 + 40 * yp + xp) * 4 + 2 + dlo
                            eng(T1[:, off:off + 1641 * 8:8],
                                T[:, st1:st1 + 1641 * 4:4])
                            st2 = (7979 + 20 * yp + xp) * 4 + 2 + dlo
                            eng(T2[:, off:off + 421 * 8:8],
                                T[:, st2:st2 + 421 * 4:4])

                # ==== 2. frontend ========================================
                qsb = qpool.tile([QP, NSLOT, DM], F32, name="qsb")
                nc.vector.memset(qsb[64:112, 2, :], 0.0)
                nc.sync.dma_start(
                    qsb[:, 0:2, :],
                    query[b][0:224].rearrange("(s r) c -> r s c", r=QP))
                nc.sync.dma_start(qsb[0:76, 2, :], query[b][224:300, :])
                refsb = qpool.tile([QP, NSLOT, 4], F32, name="refsb")
                nc.vector.memset(refsb[64:112, 2, :], 0.0)
                nc.sync.dma_start(
                    refsb[:, 0:2, :],
                    refp[b][0:224].rearrange("(s r) c -> r s c", r=QP))
                nc.sync.dma_start(refsb[0:76, 2, :], refp[b][224:300, :])

                # query^T [2][128, 336]
                qT = [qpool.tile([128, Q336], F32, name=f"qT{kt}") for kt in range(2)]
                for slot in range(NSLOT):
                    for kt in range(2):
                        pt = psS.tile([128, 128], F32, name="pss", tag="pss")
                        nc.tensor.transpose(
                            pt[:, 0:QP], qsb[:, slot, kt * 128:(kt + 1) * 128],
                            ident[0:QP, 0:QP])
                        nc.scalar.copy(qT[kt][:, slot * QP:(slot + 1) * QP],
                                       pt[:, 0:QP])
                refT = fs("refT")
                for slot in range(NSLOT):
                    pt = psS.tile([128, 128], F32, name="pss", tag="pss")
                    nc.tensor.transpose(pt[0:4, 0:QP], refsb[:, slot, :],
                                        ident[0:QP, 0:QP])
                    nc.scalar.copy(refT[0:4, slot * QP:(slot + 1) * QP],
                                   pt[0:4, 0:QP])

                # ref broadcasts: rxw, ryw, rwc, rhc  [96, 336]
                refb = []
                for m in range(4):
                    ps = psF.tile([96, Q336], F32, name="psf", tag="psf")
                    nc.tensor.matmul(ps[:], selr[:, m * 96:(m + 1) * 96], refT[0:4, :])
                    sb = fs(f"refb{m}")
                    nc.scalar.copy(sb[:], ps[:])
                    refb.append(sb)
                rxw, ryw, rwc, rhc = refb

                def head_mm(w_sb, bias_sb, cols, nm):
                    ps = psF.tile([96, Q336], F32, name="psf", tag="psf")
                    nc.tensor.matmul(ps[:], w_sb[:, 0, cols], qT[0][:],
                                     start=True, stop=False)
                    nc.tensor.matmul(ps[:], w_sb[:, 1, cols], qT[1][:],
                                     start=False, stop=False)
                    nc.tensor.matmul(ps[:], bias_sb[:, cols], ones1[:],
                                     start=False, stop=True)
                    return ps

                # attention softmax
                logit_ps = head_mm(wattn_sb, battn_sb, slice(0, 96), "logit")
                expT = fs("expT")
                nc.scalar.activation(expT[:], logit_ps[:], EXP)
                s_ps = psF.tile([96, Q336], F32, name="psf", tag="psf")
                nc.tensor.matmul(s_ps[0:8, :], sel8[:], expT[:])
                rsum = fs("rsum")
                nc.vector.reciprocal(rsum[0:8, :], s_ps[0:8, :])
                rb_ps = psF.tile([96, Q336], F32, name="psf", tag="psf")
                nc.tensor.matmul(rb_ps[:], rep8[:], rsum[0:8, :])
                attnT = fs("attnT")
                nc.vector.tensor_tensor(attnT[:], expT[:], rb_ps[:], MUL)

                offx_ps = head_mm(woff_sb, boff_sb, slice(0, 192, 2), "offx")
                offx = fs("offx")
                nc.scalar.copy(offx[:], offx_ps[:])
                offy_ps = head_mm(woff_sb, boff_sb, slice(1, 192, 2), "offy")
                offy = fs("offy")
                nc.scalar.copy(offy[:], offy_ps[:])

                # coords -> clipped ints, fracs, valids, weights
                def coord(off_sb, rXw, rWc, sfx, want_xcl):
                    t1 = fs("t1" + sfx)
                    nc.vector.tensor_tensor(t1[:], off_sb[:], rWc[:], MUL)
                    ixp = fs("ixp" + sfx)
                    nc.vector.scalar_tensor_tensor(
                        ixp[:], t1[:], CAST_BIAS, rXw[:], ADD, ADD)
                    fi = fpool.tile([96, Q336], I16, name="fi" + sfx, tag="fi")
                    nc.vector.tensor_copy(fi[:], ixp[:])
                    fxp = fs("fxp" + sfx)
                    nc.vector.tensor_copy(fxp[:], fi[:])
                    a0 = fs("a0" + sfx)
                    nc.vector.tensor_scalar(a0[:], fxp[:], SHIFT, cvs(0), MAX, MIN)
                    a1 = fs("a1" + sfx)
                    nc.vector.tensor_scalar(a1[:], fxp[:], SHIFT - 1.0, cvs(1), MAX, MIN)
                    v0 = fs("v0" + sfx)
                    nc.vector.tensor_tensor(v0[:], a0[:], fxp[:], EQ)
                    v1 = fs("v1" + sfx)
                    nc.vector.tensor_tensor(v1[:], a1[:], fxp[:], EQ)
                    fx = fs("fx" + sfx)
                    nc.vector.scalar_tensor_tensor(
                        fx[:], ixp[:], SHIFT - CAST_BIAS - 0.5, fxp[:], ADD, SUB)
                    omf = fs("omf" + sfx)
                    nc.vector.tensor_scalar(omf[:], fx[:], -1.0, 1.0, MUL, ADD)
                    w0 = fs("w0" + sfx)
                    nc.vector.tensor_tensor(w0[:], omf[:], v0[:], MUL)
                    w1 = fs("w1" + sfx)
                    nc.vector.tensor_tensor(w1[:], fx[:], v1[:], MUL)
                    xcl = None
                    if want_xcl:
                        xcl = fs("xcl")
                        nc.vector.tensor_scalar(xcl[:], fxp[:], SHIFT - 1.0,
                                                cvs(0), MAX, MIN)
                    return a0, a1, w0, w1, xcl

                _, _, wx0, wx1, xcl = coord(offx, rxw, rwc, "x", True)
                a0y, a1y, wy0, wy1, ycl = coord(offy, ryw, rhc, "y", True)

                wy0a = fs("wy0a")
                nc.vector.tensor_tensor(wy0a[:], wy0[:], attnT[:], MUL)
                wy1a = fs("wy1a")
                nc.vector.tensor_tensor(wy1a[:], wy1[:], attnT[:], MUL)

                # W8 [96, 336q, yc, xp, dlo] = wy_yc*attn * wx_xp  (dlo dup)
                W8 = wpool.tile([96, Q336, 2, 2, 2], F32, name="W8")
                for yc, wy in ((0, wy0a), (1, wy1a)):
                    for xp, wx in ((0, wx0), (1, wx1)):
                        for dlo in range(2):
                            nc.vector.tensor_tensor(
                                W8[:, :, yc, xp, dlo], wy[:], wx[:], MUL)

                # lin indices: lvl0 two y-corner streams (p<4), lvl12 one
                # y-merged stream (p>=4) -> transpose -> lin16q (int16)
                yw0 = fs("yw0")
                nc.vector.tensor_scalar(yw0[:], a0y[:], cvs(2), None, MUL)
                yw1 = fs("yw1")
                nc.vector.tensor_scalar(yw1[:], a1y[:], cvs(2), None, MUL)
                yw2 = fs("yw2")
                nc.vector.tensor_scalar(yw2[:], ycl[:], cvs(2), None, MUL)
                # cols per h: [0:4]=lf0 p<4, [4:8]=lf1 p<4, [8:16]=lf2 p>=4
                lin16q = ipool.tile([QP, NSLOT, 8, 16], I16, name="lin16q",
                                    bufs=1)
                for t, (cl, yw, p0, p1, c0) in enumerate((
                        (3, yw0, 0, 4, 0), (4, yw1, 0, 4, 4),
                        (5, yw2, 4, 12, 8))):
                    lf = fs("linf")
                    nc.vector.scalar_tensor_tensor(lf[:], xcl[:], cvs(cl), yw[:],
                                                   ADD, ADD)
                    for slot in range(NSLOT):
                        pt = psS.tile([128, 128], F32, name="pss", tag="pss")
                        nc.tensor.transpose(
                            pt[0:QP, 0:96], lf[:, slot * QP:(slot + 1) * QP],
                            ident[0:96, 0:96])
                        nc.scalar.copy(
                            lin16q[:, slot, :, c0:c0 + (p1 - p0)],
                            pt[0:QP, 0:96].rearrange(
                                "r (h p) -> r h p", h=8)[:, :, p0:p1])

                # wrapped idx tiles via DRAM round-trip (engines cannot
                # repartition; DRAM is flat).  ITs[16h+a, (sl, k, 16c)]
                nc.sync.dma_start(
                    linq_d[b], lin16q[:].rearrange("r sl h c -> r (sl h c)"))
                ITs = ipool.tile([128, NSLOT, 7, 16], I16, name="ITs",
                                 tag="ITs")
                lsrc = linq_d[b].rearrange(
                    "(k a) (sl h c) -> h a sl c k", a=16, sl=NSLOT, h=8)
                for sl in range(NSLOT):
                    for k in range(7 if sl < 2 else 5):
                        nc.sync.dma_start(ITs[:, sl, k],
                                          lsrc[:, :, sl, :, k])
                # per-gather-call contiguous idx lists
                IT0 = ipool.tile([128, NSLOT, 7, 8], I16, name="IT0", tag="IT0")
                IT1 = ipool.tile([128, NSLOT, 7, 4], I16, name="IT1", tag="IT1")
                IT2 = ipool.tile([128, NSLOT, 7, 4], I16, name="IT2", tag="IT2")
                for sl in range(NSLOT):
                    ks = 7 if sl < 2 else 5
                    nc.scalar.copy(IT0[:, sl, 0:ks], ITs[:, sl, 0:ks, 0:8])
                    nc.vector.tensor_copy(IT1[:, sl, 0:ks],
                                          ITs[:, sl, 0:ks, 8:12])
                    nc.vector.tensor_copy(IT2[:, sl, 0:ks],
                                          ITs[:, sl, 0:ks, 12:16])

                # ==== 4+5. gather + weighted reduce ======================
                oh = opool.tile([128, NSLOT, 7, 16, 2], F32, name="oh")
                import os as _os
                for slot in range(NSLOT):
                    KS = 7 if slot < 2 else 5
                    G0 = gpool.tile([128, 7, 2, 4, 64], BF16, name="G0",
                                    tag="G0")
                    G1 = gpool.tile([128, 7, 4, 16, 8], BF16, name="G1",
                                    tag="G1", bufs=1)
                    G2 = gpool.tile([128, 7, 4, 16, 8], BF16, name="G2",
                                    tag="G2", bufs=1)
                    if not _os.environ.get("KBENCH_SKIP_GATHER"):
                        nc.gpsimd.ap_gather(
                            G0[:], T[:], IT0[:, slot, 0:KS],
                            channels=128, num_elems=TBL_N, d=4,
                            num_idxs=KS * 128)
                        nc.gpsimd.ap_gather(
                            G1[:], T1[:], IT1[:, slot, 0:KS],
                            channels=128, num_elems=1648, d=8,
                            num_idxs=KS * 64)
                        nc.gpsimd.ap_gather(
                            G2[:], T2[:], IT2[:, slot, 0:KS],
                            channels=128, num_elems=432, d=8,
                            num_idxs=KS * 64)
                    else:
                        nc.vector.memset(G0[:, :, :, 0, :], 0.0)
                        nc.vector.memset(G1[:, :, 0, :, :], 0.0)
                        nc.vector.memset(G2[:, :, 0, :, :], 0.0)
                    acc = ppool.tile([128, 7, 64], F32, name="acc", tag="acc")

                    def rhs_for(yc):
                        return W8[:, slot * QP:slot * QP + 16 * KS, yc, :, :] \
                            .rearrange("c (k a) xp dl -> c k a (xp dl)", k=KS)

                    first = True

                    def step(gv, rhs):
                        nonlocal first
                        wp = psW.tile([128, 7, 16, 4], F32, name="wp",
                                      tag="wp")
                        nc.tensor.matmul(wp[:, 0:KS],
                                         seld[:, p * 128:(p + 1) * 128], rhs)
                        if first:
                            nc.vector.tensor_tensor(accv[:, 0:KS], gv,
                                                    wp[:, 0:KS], MUL)
                            first = False
                        else:
                            # product in f32 scratch: keeps bf16 rounding
                            # to the table values only
                            prod = ppool.tile([128, 7, 16, 4], F32,
                                              name="prod", tag="prod")
                            nc.vector.tensor_tensor(prod[:, 0:KS], gv,
                                                    wp[:, 0:KS], MUL)
                            nc.vector.tensor_tensor(accv[:, 0:KS],
                                                    accv[:, 0:KS],
                                                    prod[:, 0:KS], ADD)

                    accv = acc[:].rearrange("c k (a e) -> c k a e", a=16)
                    for yc in range(2):
                        rhs = rhs_for(yc)
                        for p in range(4):
                            step(G0[:, 0:KS, yc, p].rearrange(
                                "c k (a e) -> c k a e", a=16), rhs)
                    for g, G12 in ((1, G1), (2, G2)):
                        for pp in range(4):
                            p = 4 * g + pp
                            for yp in range(2):
                                gv = G12[:, 0:KS, pp, :,
                                         4 * yp:4 * yp + 4]
                                step(gv, rhs_for(yp))
                    accw = acc[:].rearrange("c k (a xp dl) -> c k a xp dl",
                                            xp=2, dl=2)
                    nc.vector.tensor_tensor(
                        oh[:, slot, 0:KS], accw[:, 0:KS, :, 0, :],
                        accw[:, 0:KS, :, 1, :], ADD)

                # ==== 6. output transpose + store ========================
                nc.vector.memset(oh[:, 2, 5:7], 0.0)
                osb = opool.tile([QP, NSLOT, DM], F32, name="osb")
                for slot in range(NSLOT):
                    for dlo in range(2):
                        pt = psS.tile([128, 128], F32, name="pss", tag="pss")
                        nc.tensor.transpose(
                            pt[0:QP, :], oh[:, slot, :, :, dlo], ident[:])
                        nc.scalar.copy(osb[:, slot, dlo::2], pt[0:QP, :])

                nc.sync.dma_start(
                    out_d[b][0:224].rearrange("(s r) c -> r s c", r=QP),
                    osb[:, 0:2, :])
                nc.sync.dma_start(out_d[b][224:300, :], osb[0:76, 2, :])
    return nc


_CACHE = {}


def _get_nc(repeat=1):
    key = ("nc", repeat)
    if key not in _CACHE:
        nc = bacc.Bacc("TRN2", target_bir_lowering=False, debug=False,
                       enable_asserts=False)
        emit(nc, repeat=repeat)
        nc.compile()
        _CACHE[key] = nc
    return _CACHE[key]


def _in_maps(query, reference_points, input_flatten, W_off, b_off, W_attn,
             b_attn):
    query = np.asarray(query, np.float32)
    refp = np.asarray(reference_points, np.float32).reshape(B, LQ, 4)
    value = np.asarray(input_flatten, np.float32)
    consts = make_consts()
    in_maps = []
    for c in range(N_CORES):
        sl = slice(c * BPC, (c + 1) * BPC)
        in_maps.append({
            "value": value[sl], "query": query[sl], "refp": refp[sl],
            "woff": np.asarray(W_off, np.float32),
            "wattn": np.asarray(W_attn, np.float32),
            "boff": np.asarray(b_off, np.float32).reshape(1, 192),
            "battn": np.asarray(b_attn, np.float32).reshape(1, 96),
            **{k: v for k, v in consts.items()},
        })
    return in_maps


def kernel(query, reference_points, input_flatten, W_off, b_off, W_attn, b_attn):
    in_maps = _in_maps(query, reference_points, input_flatten, W_off, b_off,
                       W_attn, b_attn)
    res = run_bass_kernel_spmd(_get_nc(), in_maps, core_ids=list(range(N_CORES)))
    out = np.concatenate([r["out"] for r in res.results], axis=0)
    return out.astype(np.float32)


if __name__ == "__main__":
    import reference
    inputs = reference.setup_inputs()
    inputs = {k: np.asarray(v) for k, v in inputs.items()}
    got = kernel(**inputs)
    exp = np.asarray(reference.reference(**inputs))
    err = np.abs(got - exp).max() / np.abs(exp).max()
    print("Relative error:", err)


# revision 18
# speedup vs baseline: 3.5427x; 1.0758x over previous
"""DFine MultiScale Deformable Attention — Trainium2 Bass kernel (v2).

Full inputs in, full outputs out. Data-parallel over batch: 32 batches
split 4-per-core across 8 NeuronCores (same SPMD program, per-core
input shards).

v2 redesign vs v1: the bottleneck was GPSIMD ap_gather read-command
overhead (~102 cyc per 4-index command, ReadOverlap=0 on trn2).  v1
gathered one f32 per (query, point, corner) index: 129K indices/batch.
v2 packs the value table as bf16 with d=4 per entry = (x-pair, d_lo):
  T[ch2=(8h,16dhi), px', xp, dlo] = V[px'-1+xp, h*32+dhi*2+dlo]
so ONE index serves a (q, point, y-corner): both x-corners and both
low d-channels arrive in one 8-byte fetch.  32256 indices/batch (4x
fewer commands, 4x more payload per command).  One table serves all 8
heads (Q7 core h owns head h's 16 partitions).

Pipeline per batch:
  1. DMA value [8400,256] f32 -> SBUF; PE-transpose with stride-2
     column selection (dlo split); strided bf16 copies into T with the
     x-pair duplication (entry px' holds pixels px'-1 and px').
  2. Frontend in transposed layout [(h,p)=96 part, q=336 free]:
     offsets/attn matmuls, softmax, bilinear coords.  Per (q,h,p) TWO
     gather indices (y0, y1 rows): px' = base + yc*W + x0c + 1, with
     x0c in [-1, W-1] (left-edge handled by the +1 table shift).
  3. idx wrap via DRAM round-trip -> IT[128=(h,a16), slot, yc, p, k].
  4. Per slot: ONE ap_gather (num_idxs=2688, d=4) -> G[128, yc, p, k,
     a, (xp,dlo)] bf16.
  5. Weights: W8[96,(yc,q,xp,dlo)] = attn*wy*wx; per (yc,p) PE
     broadcast (seld) to [(h,dhi), k,a,xp,dlo]; DVE multiply-accum
     over (yc,p); xp-fold -> oh[128=(h,dhi), q, dlo].
  6. PE-transpose back to q-partitions (per dlo, stride-2 merge), DMA.
"""

import numpy as np

import concourse.bass as bass
import concourse.tile as tile
from concourse import bacc, mybir, library_config
from concourse.bass_utils import run_bass_kernel_spmd

F32 = mybir.dt.float32
BF16 = mybir.dt.bfloat16
I16 = mybir.dt.int16

# Problem constants (hardcoded per task spec)
B, LQ, DM, NH, HD = 32, 300, 256, 8, 32
NP_TOT = 12                      # points per (q, h)
LVL_W = [80, 40, 20]             # square levels
LVL_BASE = [0, 6400, 8000]
S = 8400
S_PAD = 8448                     # 66 * 128
TBL_N = 8464                     # table entries (px' in [0, 8448]), padded
N_CORES = 8
BPC = B // N_CORES               # batches per core
SHIFT = 64.0                     # floor-shift (positive before cast)
CAST_BIAS = SHIFT - 1.0          # HW float->int16 rounds to nearest

# q layout: 3 slots of 112 partitions; q = 112*slot + r;  336 q-slots (300 real)
QP, NSLOT, Q336 = 112, 3, 336
NIDX_SLOT = 2 * 12 * 7 * 16      # 2688 indices per slot (yc, p, k, a)
IDXC_SLOT = NIDX_SLOT // 16      # 168 idx cols per slot


def _wl(p):
    return float(LVL_W[p // 4])


def _base(p):
    return float(LVL_BASE[p // 4])


def make_consts():
    c = {}
    c["ident"] = np.eye(128, dtype=np.float32)
    # SELD: 12 matrices [96, 128] (p-major); SELD_p[(h',p'), h*16+dhi] =
    # (p'==p && h'==h)
    seld = np.zeros((96, 12 * 128), np.float32)
    for h in range(8):
        for p in range(12):
            for dhi in range(16):
                seld[h * 12 + p, p * 128 + h * 16 + dhi] = 1.0
    c["seld"] = seld
    sel8 = np.zeros((96, 8), np.float32)
    rep8 = np.zeros((8, 96), np.float32)
    for h in range(8):
        for p in range(12):
            sel8[h * 12 + p, h] = 1.0
            rep8[h, h * 12 + p] = 1.0
    c["sel8"] = sel8
    c["rep8"] = rep8
    # SELR: 4 matrices [4, 96]: rows of refT = (x, y, w, h)
    selr = np.zeros((4, 4 * 96), np.float32)
    for hp in range(96):
        p = hp % 12
        w = _wl(p)
        selr[0, 0 * 96 + hp] = w            # RXW: ref_x * W
        selr[1, 1 * 96 + hp] = w            # RYW: ref_y * W
        selr[2, 2 * 96 + hp] = 0.125 * w    # RWC: ref_w * pscale*0.5*W
        selr[3, 3 * 96 + hp] = 0.125 * w    # RHC
    c["selr"] = selr
    c["ones1"] = np.ones((1, Q336), np.float32)
    # per-partition const vectors [96, 8]
    cv = np.zeros((96, 8), np.float32)
    for hp in range(96):
        p = hp % 12
        w, base = _wl(p), _base(p)
        cv[hp, 0] = w - 1.0 + SHIFT          # XMAX  (clip for x0/y0)
        cv[hp, 1] = w - 2.0 + SHIFT          # XM63  (clip for x1/y1 pre +1)
        cv[hp, 2] = w                        # Wv
        # lin0 = xcl + C0 + a0y*W ; px' = base + y0c*W + x0c + 1
        cv[hp, 3] = base + 1.0 - SHIFT * w - SHIFT           # C0
        cv[hp, 4] = base + 1.0 + w - SHIFT * w - SHIFT       # C1 (y1 = a1y+1)
        # lvl12 y-merged: e = (y+1)*W + x0c + 1 = ycl*W + xcl + C2
        cv[hp, 5] = w - SHIFT * w - SHIFT + 1.0              # C2
    c["cv"] = cv
    return c


def emit(nc, repeat=1):
    """Emit the full per-core program (BPC batches, x repeat)."""
    # ---- DRAM tensors -------------------------------------------------
    value = nc.dram_tensor("value", [BPC, S, DM], F32, kind="ExternalInput").ap()
    query = nc.dram_tensor("query", [BPC, LQ, DM], F32, kind="ExternalInput").ap()
    refp = nc.dram_tensor("refp", [BPC, LQ, 4], F32, kind="ExternalInput").ap()
    woff = nc.dram_tensor("woff", [DM, 192], F32, kind="ExternalInput").ap()
    wattn = nc.dram_tensor("wattn", [DM, 96], F32, kind="ExternalInput").ap()
    boff = nc.dram_tensor("boff", [1, 192], F32, kind="ExternalInput").ap()
    battn = nc.dram_tensor("battn", [1, 96], F32, kind="ExternalInput").ap()
    ident_d = nc.dram_tensor("ident", [128, 128], F32, kind="ExternalInput").ap()
    seld_d = nc.dram_tensor("seld", [96, 12 * 128], F32, kind="ExternalInput").ap()
    sel8_d = nc.dram_tensor("sel8", [96, 8], F32, kind="ExternalInput").ap()
    rep8_d = nc.dram_tensor("rep8", [8, 96], F32, kind="ExternalInput").ap()
    selr_d = nc.dram_tensor("selr", [4, 4 * 96], F32, kind="ExternalInput").ap()
    ones1_d = nc.dram_tensor("ones1", [1, Q336], F32, kind="ExternalInput").ap()
    cv_d = nc.dram_tensor("cv", [96, 8], F32, kind="ExternalInput").ap()
    out_d = nc.dram_tensor("out", [BPC, LQ, DM], F32, kind="ExternalOutput").ap()
    linq_d = nc.dram_tensor("linq", [BPC, QP, NSLOT * 8 * 16], I16,
                            kind="Internal").ap()

    MUL, ADD, SUB, MAX, MIN, EQ = (
        mybir.AluOpType.mult, mybir.AluOpType.add, mybir.AluOpType.subtract,
        mybir.AluOpType.max, mybir.AluOpType.min, mybir.AluOpType.is_equal)
    EXP = mybir.ActivationFunctionType.Exp

    with tile.TileContext(nc) as tc:
        import contextlib
        ctx = contextlib.ExitStack()
        with ctx:
            cpool = ctx.enter_context(tc.tile_pool(name="consts", bufs=1))
            spool = ctx.enter_context(tc.tile_pool(name="staged", bufs=1))
            tpool = ctx.enter_context(tc.tile_pool(name="tables", bufs=1))
            gpool = ctx.enter_context(tc.tile_pool(name="gath", bufs=2))
            fpool = ctx.enter_context(tc.tile_pool(name="front", bufs=13))
            wpool = ctx.enter_context(tc.tile_pool(name="w8", bufs=1))
            ppool = ctx.enter_context(tc.tile_pool(name="ptree", bufs=2))
            opool = ctx.enter_context(tc.tile_pool(name="outsb", bufs=1))
            ipool = ctx.enter_context(tc.tile_pool(name="idx", bufs=2))
            qpool = ctx.enter_context(tc.tile_pool(name="qt", bufs=1))
            psT = ctx.enter_context(tc.tile_pool(name="psT", bufs=3, space="PSUM"))
            psS = ctx.enter_context(tc.tile_pool(name="psS", bufs=2, space="PSUM"))
            psW = ctx.enter_context(tc.tile_pool(name="psW", bufs=2, space="PSUM"))
            psF = ctx.enter_context(tc.tile_pool(name="psF", bufs=1, space="PSUM"))

            nc.gpsimd.load_library(library_config.ap_gather)

            # ---- load constants -----------------------------------------
            def ld(dst, src):
                nc.sync.dma_start(dst, src)

            ident = cpool.tile([128, 128], F32, name="ident")
            ld(ident[:], ident_d)
            seld = cpool.tile([96, 12 * 128], F32, name="seld")
            ld(seld[:], seld_d)
            sel8 = cpool.tile([96, 8], F32, name="sel8")
            ld(sel8[:], sel8_d)
            rep8 = cpool.tile([8, 96], F32, name="rep8")
            ld(rep8[:], rep8_d)
            selr = cpool.tile([4, 4 * 96], F32, name="selr")
            ld(selr[:], selr_d)
            ones1 = cpool.tile([1, Q336], F32, name="ones1")
            ld(ones1[:], ones1_d)
            cv = cpool.tile([96, 8], F32, name="cv")
            ld(cv[:], cv_d)
            woff_sb = cpool.tile([128, 2, 192], F32, name="woff_sb")
            ld(woff_sb[:], woff.rearrange("(kt p) m -> p kt m", p=128))
            wattn_sb = cpool.tile([128, 2, 96], F32, name="wattn_sb")
            ld(wattn_sb[:], wattn.rearrange("(kt p) m -> p kt m", p=128))
            boff_sb = cpool.tile([1, 192], F32, name="boff_sb")
            ld(boff_sb[:], boff)
            battn_sb = cpool.tile([1, 96], F32, name="battn_sb")
            ld(battn_sb[:], battn)

            def cvs(k):   # [96,1] scalar AP
                return cv[:, k:k + 1]

            def fs(nm):   # shared-tag frontend scratch [96, 336] f32
                return fpool.tile([96, Q336], F32, name=nm, tag="fs")

            for b4 in range(BPC * repeat):
                b = b4 % BPC
                # ==== 2. frontend ========================================
                qsb = qpool.tile([QP, NSLOT, DM], F32, name="qsb")
                nc.vector.memset(qsb[64:112, 2, :], 0.0)
                nc.sync.dma_start(
                    qsb[:, 0:2, :],
                    query[b][0:224].rearrange("(s r) c -> r s c", r=QP))
                nc.sync.dma_start(qsb[0:76, 2, :], query[b][224:300, :])
                refsb = qpool.tile([QP, NSLOT, 4], F32, name="refsb")
                nc.vector.memset(refsb[64:112, 2, :], 0.0)
                nc.sync.dma_start(
                    refsb[:, 0:2, :],
                    refp[b][0:224].rearrange("(s r) c -> r s c", r=QP))
                nc.sync.dma_start(refsb[0:76, 2, :], refp[b][224:300, :])

                # query^T [2][128, 336]
                qT = [qpool.tile([128, Q336], F32, name=f"qT{kt}") for kt in range(2)]
                for slot in range(NSLOT):
                    for kt in range(2):
                        pt = psS.tile([128, 128], F32, name="pss", tag="pss")
                        nc.tensor.transpose(
                            pt[:, 0:QP], qsb[:, slot, kt * 128:(kt + 1) * 128],
                            ident[0:QP, 0:QP])
                        nc.scalar.copy(qT[kt][:, slot * QP:(slot + 1) * QP],
                                       pt[:, 0:QP])
                refT = fs("refT")
                for slot in range(NSLOT):
                    pt = psS.tile([128, 128], F32, name="pss", tag="pss")
                    nc.tensor.transpose(pt[0:4, 0:QP], refsb[:, slot, :],
                                        ident[0:QP, 0:QP])
                    nc.scalar.copy(refT[0:4, slot * QP:(slot + 1) * QP],
                                   pt[0:4, 0:QP])

                # ref broadcasts: rxw, ryw, rwc, rhc  [96, 336]
                refb = []
                for m in range(4):
                    ps = psF.tile([96, Q336], F32, name="psf", tag="psf")
                    nc.tensor.matmul(ps[:], selr[:, m * 96:(m + 1) * 96], refT[0:4, :])
                    sb = fs(f"refb{m}")
                    nc.scalar.copy(sb[:], ps[:])
                    refb.append(sb)
                rxw, ryw, rwc, rhc = refb

                def head_mm(w_sb, bias_sb, cols, nm):
                    ps = psF.tile([96, Q336], F32, name="psf", tag="psf")
                    nc.tensor.matmul(ps[:], w_sb[:, 0, cols], qT[0][:],
                                     start=True, stop=False)
                    nc.tensor.matmul(ps[:], w_sb[:, 1, cols], qT[1][:],
                                     start=False, stop=False)
                    nc.tensor.matmul(ps[:], bias_sb[:, cols], ones1[:],
                                     start=False, stop=True)
                    return ps

                # attention softmax
                logit_ps = head_mm(wattn_sb, battn_sb, slice(0, 96), "logit")
                expT = fs("expT")
                nc.scalar.activation(expT[:], logit_ps[:], EXP)
                s_ps = psF.tile([96, Q336], F32, name="psf", tag="psf")
                nc.tensor.matmul(s_ps[0:8, :], sel8[:], expT[:])
                rsum = fs("rsum")
                nc.vector.reciprocal(rsum[0:8, :], s_ps[0:8, :])
                rb_ps = psF.tile([96, Q336], F32, name="psf", tag="psf")
                nc.tensor.matmul(rb_ps[:], rep8[:], rsum[0:8, :])
                attnT = fs("attnT")
                nc.vector.tensor_tensor(attnT[:], expT[:], rb_ps[:], MUL)

                offx_ps = head_mm(woff_sb, boff_sb, slice(0, 192, 2), "offx")
                offx = fs("offx")
                nc.scalar.copy(offx[:], offx_ps[:])
                offy_ps = head_mm(woff_sb, boff_sb, slice(1, 192, 2), "offy")
                offy = fs("offy")
                nc.scalar.copy(offy[:], offy_ps[:])

                # coords -> clipped ints, fracs, valids, weights
                def coord(off_sb, rXw, rWc, sfx, want_xcl):
                    t1 = fs("t1" + sfx)
                    nc.vector.tensor_tensor(t1[:], off_sb[:], rWc[:], MUL)
                    ixp = fs("ixp" + sfx)
                    nc.vector.scalar_tensor_tensor(
                        ixp[:], t1[:], CAST_BIAS, rXw[:], ADD, ADD)
                    fi = fpool.tile([96, Q336], I16, name="fi" + sfx, tag="fi")
                    nc.vector.tensor_copy(fi[:], ixp[:])
                    fxp = fs("fxp" + sfx)
                    nc.vector.tensor_copy(fxp[:], fi[:])
                    a0 = fs("a0" + sfx)
                    nc.vector.tensor_scalar(a0[:], fxp[:], SHIFT, cvs(0), MAX, MIN)
                    a1 = fs("a1" + sfx)
                    nc.vector.tensor_scalar(a1[:], fxp[:], SHIFT - 1.0, cvs(1), MAX, MIN)
                    v0 = fs("v0" + sfx)
                    nc.vector.tensor_tensor(v0[:], a0[:], fxp[:], EQ)
                    v1 = fs("v1" + sfx)
                    nc.vector.tensor_tensor(v1[:], a1[:], fxp[:], EQ)
                    fx = fs("fx" + sfx)
                    nc.vector.scalar_tensor_tensor(
                        fx[:], ixp[:], SHIFT - CAST_BIAS - 0.5, fxp[:], ADD, SUB)
                    omf = fs("omf" + sfx)
                    nc.vector.tensor_scalar(omf[:], fx[:], -1.0, 1.0, MUL, ADD)
                    w0 = fs("w0" + sfx)
                    nc.vector.tensor_tensor(w0[:], omf[:], v0[:], MUL)
                    w1 = fs("w1" + sfx)
                    nc.vector.tensor_tensor(w1[:], fx[:], v1[:], MUL)
                    xcl = None
                    if want_xcl:
                        xcl = fs("xcl")
                        nc.vector.tensor_scalar(xcl[:], fxp[:], SHIFT - 1.0,
                                                cvs(0), MAX, MIN)
                    return a0, a1, w0, w1, xcl

                _, _, wx0, wx1, xcl = coord(offx, rxw, rwc, "x", True)
                a0y, a1y, wy0, wy1, ycl = coord(offy, ryw, rhc, "y", True)

                wy0a = fs("wy0a")
                nc.vector.tensor_tensor(wy0a[:], wy0[:], attnT[:], MUL)
                wy1a = fs("wy1a")
                nc.vector.tensor_tensor(wy1a[:], wy1[:], attnT[:], MUL)

                # W8 [96, 336q, yc, xp, dlo] = wy_yc*attn * wx_xp  (dlo dup)
                W8 = wpool.tile([96, Q336, 2, 2, 2], F32, name="W8")
                for yc, wy in ((0, wy0a), (1, wy1a)):
                    for xp, wx in ((0, wx0), (1, wx1)):
                        for dlo in range(2):
                            nc.vector.tensor_tensor(
                                W8[:, :, yc, xp, dlo], wy[:], wx[:], MUL)

                # lin indices: lvl0 two y-corner streams (p<4), lvl12 one
                # y-merged stream (p>=4) -> transpose -> lin16q (int16)
                yw0 = fs("yw0")
                nc.vector.tensor_scalar(yw0[:], a0y[:], cvs(2), None, MUL)
                yw1 = fs("yw1")
                nc.vector.tensor_scalar(yw1[:], a1y[:], cvs(2), None, MUL)
                yw2 = fs("yw2")
                nc.vector.tensor_scalar(yw2[:], ycl[:], cvs(2), None, MUL)
                # cols per h: [0:4]=lf0 p<4, [4:8]=lf1 p<4, [8:16]=lf2 p>=4
                lin16q = ipool.tile([QP, NSLOT, 8, 16], I16, name="lin16q",
                                    bufs=1)
                for t, (cl, yw, p0, p1, c0) in enumerate((
                        (3, yw0, 0, 4, 0), (4, yw1, 0, 4, 4),
                        (5, yw2, 4, 12, 8))):
                    lf = fs("linf")
                    nc.vector.scalar_tensor_tensor(lf[:], xcl[:], cvs(cl), yw[:],
                                                   ADD, ADD)
                    for slot in range(NSLOT):
                        pt = psS.tile([128, 128], F32, name="pss", tag="pss")
                        nc.tensor.transpose(
                            pt[0:QP, 0:96], lf[:, slot * QP:(slot + 1) * QP],
                            ident[0:96, 0:96])
                        nc.scalar.copy(
                            lin16q[:, slot, :, c0:c0 + (p1 - p0)],
                            pt[0:QP, 0:96].rearrange(
                                "r (h p) -> r h p", h=8)[:, :, p0:p1])

                # wrapped idx tiles via DRAM round-trip (engines cannot
                # repartition; DRAM is flat).  ITs[16h+a, (sl, k, 16c)]
                nc.sync.dma_start(
                    linq_d[b], lin16q[:].rearrange("r sl h c -> r (sl h c)"))
                ITs = ipool.tile([128, NSLOT, 7, 16], I16, name="ITs",
                                 tag="ITs")
                lsrc = linq_d[b].rearrange(
                    "(k a) (sl h c) -> h a sl c k", a=16, sl=NSLOT, h=8)
                for sl in range(NSLOT):
                    for k in range(7 if sl < 2 else 5):
                        nc.sync.dma_start(ITs[:, sl, k],
                                          lsrc[:, :, sl, :, k])
                # per-gather-call contiguous idx lists
                IT0 = ipool.tile([128, NSLOT, 7, 8], I16, name="IT0", tag="IT0")
                IT1 = ipool.tile([128, NSLOT, 7, 4], I16, name="IT1", tag="IT1")
                IT2 = ipool.tile([128, NSLOT, 7, 4], I16, name="IT2", tag="IT2")
                for sl in range(NSLOT):
                    ks = 7 if sl < 2 else 5
                    nc.scalar.copy(IT0[:, sl, 0:ks], ITs[:, sl, 0:ks, 0:8])
                    nc.vector.tensor_copy(IT1[:, sl, 0:ks],
                                          ITs[:, sl, 0:ks, 8:12])
                    nc.vector.tensor_copy(IT2[:, sl, 0:ks],
                                          ITs[:, sl, 0:ks, 12:16])

                # ==== 1. value staging -> packed bf16 table ==============
                # T[ch2, px'*4 + xp*2 + dlo] = V[px'-1+xp, h*32+dhi*2+dlo]
                T = tpool.tile([128, TBL_N * 4], BF16, name="T", tag="T")
                nc.vector.memset(T[:, 0:2], 0.0)   # entry 0, xp=0 slots
                # [128, 65, 256] view of the first 8320 pixels
                vb = value[b][0:8320].rearrange("(so sp) ch -> sp so ch", sp=128)

                def load_piece(j):
                    if j < 8:
                        pc = spool.tile([128, 8, 256], F32, name="piece", tag="piece")
                        nc.sync.dma_start(pc[:], vb[:, 8 * j:8 * j + 8, :])
                    else:
                        pc = spool.tile([128, 8, 256], F32, name="piece", tag="piece")
                        nc.sync.dma_start(pc[:, 0, :], vb[:, 64, :])
                        nc.vector.memset(pc[64:128, 1, :], 0.0)
                        nc.sync.dma_start(pc[0:80, 1, :], value[b][8320:8400, :])
                    return pc

                pieces = {}
                for g in range(17):           # groups of 4 chunks (66 = 16*4+2)
                    gsz = 4 if g < 16 else 2
                    px0 = 512 * g
                    for dlo in range(2):
                        pt = psT.tile([128, 512], F32, name="tpose", tag="tpose")
                        for u in range(gsz):
                            so = g * 4 + u
                            j = so // 8 if so < 64 else 8
                            if j not in pieces:
                                pieces[j] = load_piece(j)
                            loc = so % 8 if so < 64 else so - 64
                            src = pieces[j][:, loc, dlo::2]
                            nc.tensor.transpose(pt[:, u * 128:(u + 1) * 128],
                                                src, ident[:])
                        n = gsz * 128
                        for xp in range(2):
                            st = 4 * px0 + (4 - 2 * xp) + dlo
                            dst = T[:, st:st + 4 * n:4]
                            if xp == 0:
                                nc.scalar.copy(dst, pt[:, 0:n])
                            else:
                                nc.vector.tensor_copy(dst, pt[:, 0:n])

                # ---- y-merged d=8 tables for levels 1, 2 (from T) -------
                # T1[e*8+yp*4+xp*2+dlo] = V[e+6359+40*yp+xp] (lvl1 pixel
                # (y+yp, x0c+xp), e=(y+1)*40+x0c+1); V[g] = T[g*4+2+dlo]
                T1 = tpool.tile([128, 1648 * 8], BF16, name="T1", tag="T1")
                T2 = tpool.tile([128, 432 * 8], BF16, name="T2", tag="T2")
                for yp in range(2):
                    for xp in range(2):
                        for dlo in range(2):
                            off = yp * 4 + xp * 2 + dlo
                            eng = nc.scalar.copy if yp == 0 else nc.vector.tensor_copy
                            st1 = (6359 + 40 * yp + xp) * 4 + 2 + dlo
                            eng(T1[:, off:off + 1641 * 8:8],
                                T[:, st1:st1 + 1641 * 4:4])
                            st2 = (7979 + 20 * yp + xp) * 4 + 2 + dlo
                            eng(T2[:, off:off + 421 * 8:8],
                                T[:, st2:st2 + 421 * 4:4])

                # ==== 4+5. gather + weighted reduce ======================
                oh = opool.tile([128, NSLOT, 7, 16, 2], F32, name="oh")
                import os as _os
                for slot in range(NSLOT):
                    KS = 7 if slot < 2 else 5
                    G0 = gpool.tile([128, 7, 2, 4, 64], BF16, name="G0",
                                    tag="G0")
                    G1 = gpool.tile([128, 7, 4, 16, 8], BF16, name="G1",
                                    tag="G1", bufs=1)
                    G2 = gpool.tile([128, 7, 4, 16, 8], BF16, name="G2",
                                    tag="G2", bufs=1)
                    if not _os.environ.get("KBENCH_SKIP_GATHER"):
                        nc.gpsimd.ap_gather(
                            G0[:], T[:], IT0[:, slot, 0:KS],
                            channels=128, num_elems=TBL_N, d=4,
                            num_idxs=KS * 128)
                        nc.gpsimd.ap_gather(
                            G1[:], T1[:], IT1[:, slot, 0:KS],
                            channels=128, num_elems=1648, d=8,
                            num_idxs=KS * 64)
                        nc.gpsimd.ap_gather(
                            G2[:], T2[:], IT2[:, slot, 0:KS],
                            channels=128, num_elems=432, d=8,
                            num_idxs=KS * 64)
                    else:
                        nc.vector.memset(G0[:, :, :, 0, :], 0.0)
                        nc.vector.memset(G1[:, :, 0, :, :], 0.0)
                        nc.vector.memset(G2[:, :, 0, :, :], 0.0)
                    acc = ppool.tile([128, 7, 64], F32, name="acc", tag="acc")

                    def rhs_for(yc):
                        return W8[:, slot * QP:slot * QP + 16 * KS, yc, :, :] \
                            .rearrange("c (k a) xp dl -> c k a (xp dl)", k=KS)

                    first = True

                    def step(gv, rhs):
                        nonlocal first
                        wp = psW.tile([128, 7, 16, 4], F32, name="wp",
                                      tag="wp")
                        nc.tensor.matmul(wp[:, 0:KS],
                                         seld[:, p * 128:(p + 1) * 128], rhs)
                        if first:
                            nc.vector.tensor_tensor(accv[:, 0:KS], gv,
                                                    wp[:, 0:KS], MUL)
                            first = False
                        else:
                            # product in f32 scratch: keeps bf16 rounding
                            # to the table values only
                            prod = ppool.tile([128, 7, 16, 4], F32,
                                              name="prod", tag="prod")
                            nc.vector.tensor_tensor(prod[:, 0:KS], gv,
                                                    wp[:, 0:KS], MUL)
                            nc.vector.tensor_tensor(accv[:, 0:KS],
                                                    accv[:, 0:KS],
                                                    prod[:, 0:KS], ADD)

                    accv = acc[:].rearrange("c k (a e) -> c k a e", a=16)
                    for yc in range(2):
                        rhs = rhs_for(yc)
                        for p in range(4):
                            step(G0[:, 0:KS, yc, p].rearrange(
                                "c k (a e) -> c k a e", a=16), rhs)
                    for g, G12 in ((1, G1), (2, G2)):
                        for pp in range(4):
                            p = 4 * g + pp
                            for yp in range(2):
                                gv = G12[:, 0:KS, pp, :,
                                         4 * yp:4 * yp + 4]
                                step(gv, rhs_for(yp))
                    accw = acc[:].rearrange("c k (a xp dl) -> c k a xp dl",
                                            xp=2, dl=2)
                    nc.vector.tensor_tensor(
                        oh[:, slot, 0:KS], accw[:, 0:KS, :, 0, :],
                        accw[:, 0:KS, :, 1, :], ADD)

                # ==== 6. output transpose + store ========================
                nc.vector.memset(oh[:, 2, 5:7], 0.0)
                osb = opool.tile([QP, NSLOT, DM], F32, name="osb")
                for slot in range(NSLOT):
                    for dlo in range(2):
                        pt = psS.tile([128, 128], F32, name="pss", tag="pss")
                        nc.tensor.transpose(
                            pt[0:QP, :], oh[:, slot, :, :, dlo], ident[:])
                        nc.scalar.copy(osb[:, slot, dlo::2], pt[0:QP, :])

                nc.sync.dma_start(
                    out_d[b][0:224].rearrange("(s r) c -> r s c", r=QP),
                    osb[:, 0:2, :])
                nc.sync.dma_start(out_d[b][224:300, :], osb[0:76, 2, :])
    return nc


_CACHE = {}


def _get_nc(repeat=1):
    key = ("nc", repeat)
    if key not in _CACHE:
        nc = bacc.Bacc("TRN2", target_bir_lowering=False, debug=False,
                       enable_asserts=False)
        emit(nc, repeat=repeat)
        nc.compile()
        _CACHE[key] = nc
    return _CACHE[key]


def _in_maps(query, reference_points, input_flatten, W_off, b_off, W_attn,
             b_attn):
    query = np.asarray(query, np.float32)
    refp = np.asarray(reference_points, np.float32).reshape(B, LQ, 4)
    value = np.asarray(input_flatten, np.float32)
    consts = make_consts()
    in_maps = []
    for c in range(N_CORES):
        sl = slice(c * BPC, (c + 1) * BPC)
        in_maps.append({
            "value": value[sl], "query": query[sl], "refp": refp[sl],
            "woff": np.asarray(W_off, np.float32),
            "wattn": np.asarray(W_attn, np.float32),
            "boff": np.asarray(b_off, np.float32).reshape(1, 192),
            "battn": np.asarray(b_attn, np.float32).reshape(1, 96),
            **{k: v for k, v in consts.items()},
        })
    return in_maps


def kernel(query, reference_points, input_flatten, W_off, b_off, W_attn, b_attn):
    in_maps = _in_maps(query, reference_points, input_flatten, W_off, b_off,
                       W_attn, b_attn)
    res = run_bass_kernel_spmd(_get_nc(), in_maps, core_ids=list(range(N_CORES)))
    out = np.concatenate([r["out"] for r in res.results], axis=0)
    return out.astype(np.float32)


if __name__ == "__main__":
    import reference
    inputs = reference.setup_inputs()
    inputs = {k: np.asarray(v) for k, v in inputs.items()}
    got = kernel(**inputs)
    exp = np.asarray(reference.reference(**inputs))
    err = np.abs(got - exp).max() / np.abs(exp).max()
    print("Relative error:", err)
